# revision 1
# baseline (speedup 1.0000x reference)
"""Trainium2 Bass kernel for nn_Euclidian (segment_reduce):

    counts/centers = segment mean of feat by label (C=100 classes)
    out[i] = || feat[i] - centers[label[i]] ||_2

Strategy (8 NeuronCores, data-parallel over N):
  pass 1: per 128-sample tile, onehot[128,100] = (iota == label); PSUM
          accumulate centers_sum[100,256] += onehot.T @ feat  (PE, f32r)
  AllReduce[100,256] across the 8 cores (tiny); centers = sums * (1/count)
          (1/count precomputed host-side from labels alone)
  pass 2: G[128,256] = onehotT.T @ centers gathers each sample's center row
          on the PE (no HBM gather traffic); onehotT built by broadcasting
          labels across partitions with a K=1 matmul + is_equal.
          dist = sqrt(sum((feat-G)^2)) via DVE subtract + ACT square-accum.

feat is read from HBM exactly twice — memory roofline.
"""

import contextlib

import numpy as np

import concourse.mybir as mybir
import concourse.tile as tile
from concourse import bacc
from concourse.bass_utils import run_bass_kernel_spmd

F32 = mybir.dt.float32
F32R = mybir.dt.float32r
I32 = mybir.dt.int32

P = 128  # partitions / samples per tile
C = 100  # num classes
D = 256  # feature dim

N_FULL = 500000
N_CORES = 8
NS = N_FULL // N_CORES  # 62500 samples per core
GROUP = 8  # tiles per feat DMA group


def _group_sizes(np_pad):
    n_tiles = np_pad // P
    groups = []
    t = 0
    while t < n_tiles:
        g = min(GROUP, n_tiles - t)
        groups.append(g)
        t += g
    return groups


def build(np_pad, num_devices=N_CORES, mode="full", group=None, sb_bufs=3, p2var="std", gbufs=4, dbufs=3, oht_eng="vector"):
    """Build the per-core SPMD program for np_pad (multiple of 128) samples.

    mode: "full" | "pass1" | "pass1nc" | "pass2" | "dma" | "loopN"
    (N repetitions of the collective-free body, for timing).
    """
    assert np_pad % P == 0
    global GROUP
    if group is not None:
        GROUP = group
    loops, loop_what = 0, "all"
    if mode.startswith("loop"):
        m = mode[4:]
        for suf in ("p1", "p2", "dma"):
            if m.endswith(suf):
                loop_what, m = suf, m[: -len(suf)]
                break
        loops = int(m)
    do_p1 = mode in ("full", "pass1", "pass1nc") or (loops and loop_what in ("all", "p1"))
    do_p2 = mode in ("full", "pass2") or (loops and loop_what in ("all", "p2"))
    do_cc = mode == "full" and num_devices > 1
    groups = _group_sizes(np_pad)

    nc = bacc.Bacc(
        "TRN2",
        target_bir_lowering=False,
        debug=False,
        enable_asserts=True,
        num_devices=num_devices,
    )

    feat_d = nc.dram_tensor("feat", [np_pad, D], F32, kind="ExternalInput")
    labp_d = nc.dram_tensor("labp", [np_pad], F32, kind="ExternalInput")  # p-major
    labf_d = nc.dram_tensor("labf", [np_pad], F32, kind="ExternalInput")  # flat
    crec_d = nc.dram_tensor("crec", [C, 1], F32, kind="ExternalInput")  # 1/max(cnt,1)
    out_d = nc.dram_tensor("given", [np_pad], F32, kind="ExternalOutput")

    with tile.TileContext(nc) as tc, contextlib.ExitStack() as ctx:
        const = ctx.enter_context(tc.tile_pool(name="const", bufs=1))
        sb1 = ctx.enter_context(tc.tile_pool(name="sb1", bufs=sb_bufs))
        oh1 = ctx.enter_context(tc.tile_pool(name="oh1", bufs=4))
        dram = ctx.enter_context(tc.tile_pool(name="dram", bufs=1, space="DRAM"))

        # ---------------- constants ----------------
        iota_i = const.tile([P, C], I32)
        nc.gpsimd.iota(iota_i[:], pattern=[[1, C]], base=0, channel_multiplier=0)
        iota_row = const.tile([P, C], F32)
        nc.vector.tensor_copy(iota_row[:], iota_i[:])

        iotac_i = const.tile([C, 1], I32)
        nc.gpsimd.iota(iotac_i[:], pattern=[[0, 1]], base=0, channel_multiplier=1)
        iota_col = const.tile([C, 1], F32)
        nc.vector.tensor_copy(iota_col[:], iotac_i[:])

        ones_f = const.tile([1, C], F32)
        nc.vector.memset(ones_f[:1, :], 1.0)
        ones_row = const.tile([1, C], F32R)
        nc.vector.tensor_copy(ones_row[:1, :], ones_f[:1, :])

        crec_sb = const.tile([C, 1], F32)
        nc.sync.dma_start(out=crec_sb[:], in_=crec_d[:, :])

        centers_r = const.tile([C, D], F32R)
        centers_ext = const.tile([C, D + 1], F32R)

        iota_pi = const.tile([P, P], I32)
        nc.gpsimd.iota(iota_pi[:], pattern=[[1, P]], base=0, channel_multiplier=0)
        iota_pf = const.tile([P, P], F32)
        nc.vector.tensor_copy(iota_pf[:], iota_pi[:])
        iotac_pi = const.tile([P, 1], I32)
        nc.gpsimd.iota(iotac_pi[:], pattern=[[0, 1]], base=0, channel_multiplier=1)
        iotac_pf = const.tile([P, 1], F32)
        nc.vector.tensor_copy(iotac_pf[:], iotac_pi[:])
        ident_r = const.tile([P, P], F32R)
        nc.vector.tensor_scalar(
            out=ident_r[:],
            in0=iota_pf[:],
            scalar1=iotac_pf[:, :1],
            scalar2=None,
            op0=mybir.AluOpType.is_equal,
        )

        n_tiles_total = np_pad // P
        tp_pad = ((n_tiles_total + 511) // 512) * 512
        persist = ctx.enter_context(tc.tile_pool(name="persist", bufs=1))
        dot_mode = p2var == "dot"
        sqf_all = persist.tile([P, tp_pad], F32, name="sqf_all") if dot_mode else None

        def emit_pass1():
            """Local segment sums -> sums_sb [C, D] (SBUF, f32)."""
            with tc.tile_pool(name="ps1", bufs=1, space="PSUM") as ps1:
                acc_ps = ps1.tile([C, D], F32, space="PSUM")
                ti = 0
                off = 0
                for g in groups:
                    w = g * P
                    feat_g = sb1.tile([P, GROUP * D], F32R, tag="feat1")
                    # partition p <- sample row off + t*128 + p
                    nc.sync.dma_start(
                        out=feat_g[:, : g * D].rearrange("p (t d) -> p t d", d=D),
                        in_=feat_d[off : off + w, :]
                        .rearrange("(p t) d -> p t d", p=P)
                        .bitcast(F32R),
                    )
                    labp_g = sb1.tile([P, GROUP], F32, tag="labp")
                    nc.sync.dma_start(
                        out=labp_g[:, :g],
                        in_=labp_d[off : off + w].rearrange("(p t) -> p t", p=P),
                    )
                    for t in range(g):
                        onehot = oh1.tile([P, C], F32R, tag="oh")
                        nc.vector.tensor_scalar(
                            out=onehot[:],
                            in0=iota_row[:],
                            scalar1=labp_g[:, t : t + 1],
                            scalar2=None,
                            op0=mybir.AluOpType.is_equal,
                        )
                        nc.tensor.matmul(
                            acc_ps[:],
                            lhsT=onehot[:],
                            rhs=feat_g[:, t * D : (t + 1) * D],
                            start=(ti == 0),
                            stop=(ti == n_tiles_total - 1),
                        )
                        if dot_mode:
                            sq1 = oh1.tile([P, D], F32, tag="sq1", bufs=dbufs)
                            nc.vector.tensor_tensor_reduce(
                                out=sq1[:],
                                in0=feat_g[:, t * D : (t + 1) * D].bitcast(F32),
                                in1=feat_g[:, t * D : (t + 1) * D].bitcast(F32),
                                scale=1.0,
                                scalar=0.0,
                                op0=mybir.AluOpType.mult,
                                op1=mybir.AluOpType.add,
                                accum_out=sqf_all[:, ti : ti + 1],
                            )
                        ti += 1
                    off += w
                sums_sb = const.tile([C, D], F32)
                nc.vector.tensor_copy(sums_sb[:], acc_ps[:])
            return sums_sb

        def emit_centers(sums_sb, collective):
            """AllReduce sums (optional) and scale by 1/count -> centers_r."""
            cc_in = dram.tile([C, D], F32)
            nc.sync.dma_start(out=cc_in[:], in_=sums_sb[:])
            if collective:
                cc_out = dram.tile([C, D], F32)
                nc.gpsimd.collective_compute(
                    "AllReduce",
                    mybir.AluOpType.add,
                    replica_groups=[list(range(num_devices))],
                    ins=[cc_in.opt()],
                    outs=[cc_out.opt()],
                )
                gsrc = cc_out
            else:
                gsrc = cc_in
            gsums_sb = const.tile([C, D], F32)
            nc.sync.dma_start(out=gsums_sb[:], in_=gsrc[:])
            if dot_mode:
                cent_f = const.tile([C, D], F32)
                nc.vector.tensor_scalar(
                    out=cent_f[:],
                    in0=gsums_sb[:],
                    scalar1=crec_sb[:, :1],
                    scalar2=None,
                    op0=mybir.AluOpType.mult,
                )
                csq_scr = const.tile([C, D], F32)
                sqc = const.tile([C, 1], F32)
                nc.scalar.activation(
                    out=csq_scr[:],
                    in_=cent_f[:],
                    func=mybir.ActivationFunctionType.Square,
                    accum_out=sqc[:, :1],
                )
                nc.vector.tensor_copy(centers_ext[:, :D], cent_f[:])
                nc.vector.tensor_copy(centers_ext[:, D : D + 1], sqc[:, :1])
            else:
                nc.vector.tensor_scalar(
                    out=centers_r[:],
                    in0=gsums_sb[:],
                    scalar1=crec_sb[:, :1],
                    scalar2=None,
                    op0=mybir.AluOpType.mult,
                )
            return gsums_sb

        def emit_pass2():
            """Distances using centers_r -> out_d."""
            with (
                tc.tile_pool(name="ps_lb", bufs=2, space="PSUM") as ps_lb,
                tc.tile_pool(name="ps_g", bufs=gbufs, space="PSUM") as ps_g,
                tc.tile_pool(name="sb2", bufs=sb_bufs) as sb2,
                tc.tile_pool(name="resp", bufs=1) as resp,
            ):
                res_all = resp.tile([P, ((n_tiles_total + 511) // 512) * 512], F32)
                tbase = 0
                off = 0
                for g in groups:
                    w = g * P
                    f2dt = F32R if p2var == "psum" else F32
                    feat_g = sb2.tile([P, GROUP * D], f2dt, tag="feat2")
                    f2src = feat_d[off : off + w, :].rearrange("(p t) d -> p t d", p=P)
                    nc.sync.dma_start(
                        out=feat_g[:, : g * D].rearrange("p (t d) -> p t d", d=D),
                        in_=f2src.bitcast(F32R) if p2var == "psum" else f2src,
                    )
                    labf_g = sb2.tile([1, GROUP * P], F32R, tag="labf")
                    if p2var != "fixedoht":
                        nc.sync.dma_start(
                            out=labf_g[:1, :w],
                            in_=labf_d[None, off : off + w].bitcast(F32R),
                        )
                    oht_g = sb2.tile([C, GROUP * P], F32R, tag="oht")
                    for h in ([] if p2var in ("nope", "fixedoht") else range(0, w, 512)):
                        hw = min(512, w - h)
                        lb_ps = ps_lb.tile([C, 512], F32, space="PSUM", tag="lb")
                        nc.tensor.matmul(
                            lb_ps[:, :hw],
                            lhsT=ones_row[:1, :],
                            rhs=labf_g[:1, h : h + hw],
                            start=True,
                            stop=True,
                        )
                        _oht_kw = (
                            dict(scalar2=-1.0, op1=mybir.AluOpType.mult)
                            if p2var == "psum"
                            else dict(scalar2=None)
                        )
                        getattr(nc, oht_eng).tensor_scalar(
                            out=oht_g[:, h : h + hw],
                            in0=lb_ps[:, :hw],
                            scalar1=iota_col[:, :1],
                            op0=mybir.AluOpType.is_equal,
                            **_oht_kw,
                        )
                    res_g = res_all[:, tbase : tbase + g]
                    for t in range(g):
                        if p2var == "psum":
                            g_ps = ps_g.tile([P, D], F32, space="PSUM", tag="g")
                            nc.tensor.matmul(
                                g_ps[:],
                                lhsT=oht_g[:, t : w : g],
                                rhs=centers_r[:],
                                start=True,
                                stop=False,
                            )
                            nc.tensor.matmul(
                                g_ps[:],
                                lhsT=ident_r[:],
                                rhs=feat_g[:, t * D : (t + 1) * D],
                                start=False,
                                stop=True,
                            )
                            nc.scalar.activation(
                                out=g_ps[:],
                                in_=g_ps[:],
                                func=mybir.ActivationFunctionType.Square,
                                accum_out=res_g[:, t : t + 1],
                            )
                            continue
                        if p2var == "fixedoht":
                            g_ps = ps_g.tile([P, D], F32, space="PSUM", tag="g")
                            nc.tensor.matmul(
                                g_ps[:],
                                lhsT=ident_r[:C, :],
                                rhs=centers_r[:],
                                start=True,
                                stop=True,
                            )
                            diff = sb2.tile([P, D], F32, tag="diff", bufs=dbufs)
                            nc.vector.tensor_tensor(
                                out=diff[:],
                                in0=feat_g[:, t * D : (t + 1) * D],
                                in1=g_ps[:],
                                op=mybir.AluOpType.subtract,
                            )
                            nc.scalar.activation(
                                out=diff[:],
                                in_=diff[:],
                                func=mybir.ActivationFunctionType.Square,
                                accum_out=res_g[:, t : t + 1],
                            )
                            continue
                        use_g = p2var in ("std", "noact")
                        if use_g:
                            g_ps = ps_g.tile([P, D], F32, space="PSUM", tag="g")
                            nc.tensor.matmul(
                                g_ps[:],
                                lhsT=oht_g[:, t : w : g],
                                rhs=centers_r[:],
                                start=True,
                                stop=True,
                            )
                        if p2var in ("std", "noact"):
                            diff = sb2.tile([P, D], F32, tag="diff", bufs=dbufs)
                            nc.vector.tensor_tensor(
                                out=diff[:],
                                in0=feat_g[:, t * D : (t + 1) * D],
                                in1=g_ps[:],
                                op=mybir.AluOpType.subtract,
                            )
                        if p2var == "std":
                            nc.scalar.activation(
                                out=diff[:],
                                in_=diff[:],
                                func=mybir.ActivationFunctionType.Square,
                                accum_out=res_g[:, t : t + 1],
                            )
                        elif p2var == "noact":
                            nc.vector.tensor_copy(res_g[:, t : t + 1], diff[:, 0:1])
                        elif p2var == "nodve":
                            sq = sb2.tile([P, D], F32, tag="sq")
                            nc.scalar.activation(
                                out=sq[:],
                                in_=feat_g[:, t * D : (t + 1) * D],
                                func=mybir.ActivationFunctionType.Square,
                                accum_out=res_g[:, t : t + 1],
                            )
                        elif p2var == "nope":
                            sq = sb2.tile([P, D], F32, tag="sq")
                            nc.scalar.activation(
                                out=sq[:],
                                in_=feat_g[:, t * D : (t + 1) * D],
                                func=mybir.ActivationFunctionType.Square,
                                accum_out=res_g[:, t : t + 1],
                            )
                    tbase += g
                    off += w
                # single sqrt over every accumulated column, then write out.
                if dot_mode:
                    nc.vector.tensor_tensor(
                        out=res_all[:, :n_tiles_total],
                        in0=res_all[:, :n_tiles_total],
                        in1=sqf_all[:, :n_tiles_total],
                        op=mybir.AluOpType.add,
                    )
                nc.scalar.activation(
                    out=res_all[:, :n_tiles_total],
                    in_=res_all[:, :n_tiles_total],
                    func=mybir.ActivationFunctionType.Sqrt,
                )
                # uniform groups: out[goff + p*g + t] = res_all[p, 8*grp + t]
                n_uni = np_pad // (GROUP * P)
                if n_uni:
                    nc.sync.dma_start(
                        out=out_d[: n_uni * GROUP * P].rearrange(
                            "(grp p t) -> p grp t", p=P, t=GROUP
                        ),
                        in_=res_all[:, : n_uni * GROUP].rearrange(
                            "p (grp t) -> p grp t", t=GROUP
                        ),
                    )
                toff = n_uni * GROUP
                soff = n_uni * GROUP * P
                for g in groups[n_uni:]:
                    nc.sync.dma_start(
                        out=out_d[soff : soff + g * P].rearrange("(p t) -> p t", p=P),
                        in_=res_all[:, toff : toff + g],
                    )
                    toff += g
                    soff += g * P

        if loops:
            if loop_what in ("all", "p2") and not do_p1:
                cfill = const.tile([C, D + 1], F32)
                nc.vector.memset(cfill[:], 0.01)
                nc.vector.tensor_copy(centers_r[:], cfill[:, :D])
                nc.vector.tensor_copy(centers_ext[:], cfill[:])
            with tc.For_i(0, loops, 1):
                if loop_what == "dma":
                    off = 0
                    for g in groups:
                        w = g * P
                        feat_g = sb1.tile([P, GROUP * D], F32, tag="featd")
                        nc.sync.dma_start(
                            out=feat_g[:, : g * D].rearrange("p (t d) -> p t d", d=D),
                            in_=feat_d[off : off + w, :].rearrange(
                                "(p t) d -> p t d", p=P
                            ),
                        )
                        off += w
                if do_p1:
                    sums_sb = emit_pass1()
                    emit_centers(sums_sb, collective=False)
                if do_p2:
                    emit_pass2()
            if loop_what == "dma":
                z = const.tile([P, 1], F32)
                nc.vector.memset(z[:], 0.0)
                nc.sync.dma_start(out=out_d[0:P, None], in_=z[:])
        elif mode == "dma":
            for _ in range(2):
                off = 0
                for g in groups:
                    w = g * P
                    feat_g = sb1.tile([P, GROUP * D], F32, tag="featd")
                    nc.sync.dma_start(
                        out=feat_g[:, : g * D].rearrange("p (t d) -> p t d", d=D),
                        in_=feat_d[off : off + w, :].rearrange("(p t) d -> p t d", p=P),
                    )
                    off += w
            z = const.tile([P, 1], F32)
            nc.vector.memset(z[:], 0.0)
            nc.sync.dma_start(out=out_d[0:P, None], in_=z[:])
        else:
            if do_p1:
                sums_sb = emit_pass1()
                gsums_sb = emit_centers(sums_sb, collective=do_cc)
                if mode in ("pass1", "pass1nc"):
                    nc.sync.dma_start(
                        out=out_d[0 : C * D].rearrange("(c d) -> c d", d=D),
                        in_=gsums_sb[:],
                    )
            elif do_p2:
                cfill = const.tile([C, D + 1], F32)
                nc.vector.memset(cfill[:], 0.01)
                nc.vector.tensor_copy(centers_r[:], cfill[:, :D])
                nc.vector.tensor_copy(centers_ext[:], cfill[:])
            if do_p2:
                emit_pass2()

    nc.compile()
    return nc


def build_nop(num_devices=N_CORES):
    """Minimal kernel (copy one tile) to measure the dispatch floor."""
    nc = bacc.Bacc(
        "TRN2",
        target_bir_lowering=False,
        debug=False,
        enable_asserts=True,
        num_devices=num_devices,
    )
    x_d = nc.dram_tensor("x", [P, P], F32, kind="ExternalInput")
    y_d = nc.dram_tensor("y", [P, P], F32, kind="ExternalOutput")
    with tile.TileContext(nc) as tc:
        with tc.tile_pool(name="sb", bufs=1) as sb:
            t = sb.tile([P, P], F32)
            nc.sync.dma_start(out=t[:], in_=x_d[:, :])
            nc.sync.dma_start(out=y_d[:, :], in_=t[:])
    nc.compile()
    return nc


def _prep_core_inputs(feat_c, lab_c, crec, np_pad):
    """Host-side shard prep: pad + layout labels; all f32."""
    ns = feat_c.shape[0]
    fpad = np.zeros((np_pad, D), dtype=np.float32)
    fpad[:ns] = feat_c
    lab = np.full((np_pad,), float(C), dtype=np.float32)  # pad label = C -> no class
    lab[:ns] = lab_c.astype(np.float32)
    return {"feat": fpad, "labp": lab, "labf": lab, "crec": crec}


_CACHE = {}


def _get_nc(np_pad, num_devices, **kw):
    key = (np_pad, num_devices, tuple(sorted(kw.items())))
    if key not in _CACHE:
        _CACHE[key] = build(np_pad, num_devices, **kw)
    return _CACHE[key]


def run(feat, label, np_pad=None, num_devices=N_CORES, trace=False, **kw):
    n = feat.shape[0]
    ns = n // num_devices
    if np_pad is None:
        np_pad = ((ns + P - 1) // P) * P
    nc = _get_nc(np_pad, num_devices, **kw)

    cnt = np.bincount(label.astype(np.int64), minlength=C)[:C]
    crec = (1.0 / np.maximum(cnt, 1)).astype(np.float32)[:, None]

    in_maps = [
        _prep_core_inputs(
            feat[c * ns : (c + 1) * ns], label[c * ns : (c + 1) * ns], crec, np_pad
        )
        for c in range(num_devices)
    ]
    res = run_bass_kernel_spmd(
        nc, in_maps, core_ids=list(range(num_devices)), trace=trace
    )
    out = np.concatenate([res.results[c]["given"][:ns] for c in range(num_devices)])
    return out, res


def kernel(feat, label):
    feat = np.asarray(feat, dtype=np.float32)
    label = np.asarray(label)
    out, _ = run(feat, label)
    return out.astype(np.float32)



# revision 16
# speedup vs baseline: 1.8760x; 1.8760x over previous
"""Trainium2 Bass kernel for nn_Euclidian (segment_reduce):

    counts/centers = segment mean of feat by label (C=100 classes)
    out[i] = || feat[i] - centers[label[i]] ||_2

Strategy (8 NeuronCores, data-parallel over N):
  pass 1: per 128-sample tile, onehot[128,100] = (iota == label); PSUM
          accumulate centers_sum[100,256] += onehot.T @ feat  (PE, f32r)
  AllReduce[100,256] across the 8 cores (tiny); centers = sums * (1/count)
          (1/count precomputed host-side from labels alone)
  pass 2: G[128,256] = onehotT.T @ centers gathers each sample's center row
          on the PE (no HBM gather traffic); onehotT built by broadcasting
          labels across partitions with a K=1 matmul + is_equal.
          dist = sqrt(sum((feat-G)^2)) via DVE subtract + ACT square-accum.

feat is read from HBM exactly twice — memory roofline.
"""

import contextlib

import numpy as np

import concourse.mybir as mybir
import concourse.tile as tile
from concourse import bacc
from concourse.bass_utils import run_bass_kernel_spmd

F32 = mybir.dt.float32
F32R = mybir.dt.float32r
I32 = mybir.dt.int32

P = 128  # partitions / samples per tile
C = 100  # num classes
D = 256  # feature dim

N_FULL = 500000
N_CORES = 8
NS = N_FULL // N_CORES  # 62500 samples per core
GROUP = 8  # tiles per feat DMA group


def _group_sizes(np_pad):
    n_tiles = np_pad // P
    groups = []
    t = 0
    while t < n_tiles:
        g = min(GROUP, n_tiles - t)
        groups.append(g)
        t += g
    return groups


def build(np_pad, num_devices=N_CORES, mode="full", group=None, sb_bufs=3, p2var="std", gbufs=4, dbufs=3, oht_eng="vector", oht1_eng="vector", p1sq="dve"):
    """Build the per-core SPMD program for np_pad (multiple of 128) samples.

    mode: "full" | "pass1" | "pass1nc" | "pass2" | "dma" | "loopN"
    (N repetitions of the collective-free body, for timing).
    """
    assert np_pad % P == 0
    global GROUP
    if group is not None:
        GROUP = group
    loops, loop_what = 0, "all"
    if mode.startswith("loop"):
        m = mode[4:]
        for suf in ("p1", "p2", "dma"):
            if m.endswith(suf):
                loop_what, m = suf, m[: -len(suf)]
                break
        loops = int(m)
    do_p1 = mode in ("full", "pass1", "pass1nc") or (loops and loop_what in ("all", "p1"))
    do_p2 = mode in ("full", "pass2") or (loops and loop_what in ("all", "p2"))
    do_cc = mode == "full" and num_devices > 1
    groups = _group_sizes(np_pad)

    nc = bacc.Bacc(
        "TRN2",
        target_bir_lowering=False,
        debug=False,
        enable_asserts=True,
        num_devices=num_devices,
    )

    feat_d = nc.dram_tensor("feat", [np_pad, D], F32, kind="ExternalInput")
    labp_d = nc.dram_tensor("labp", [np_pad], F32, kind="ExternalInput")  # p-major
    labf_d = nc.dram_tensor("labf", [np_pad], F32, kind="ExternalInput")  # flat
    crec_d = nc.dram_tensor("crec", [C, 1], F32, kind="ExternalInput")  # 1/max(cnt,1)
    out_d = nc.dram_tensor("given", [np_pad], F32, kind="ExternalOutput")

    with tile.TileContext(nc) as tc, contextlib.ExitStack() as ctx:
        const = ctx.enter_context(tc.tile_pool(name="const", bufs=1))
        sb1 = ctx.enter_context(tc.tile_pool(name="sb1", bufs=sb_bufs))
        oh1 = ctx.enter_context(tc.tile_pool(name="oh1", bufs=4))
        dram = ctx.enter_context(tc.tile_pool(name="dram", bufs=1, space="DRAM"))

        # ---------------- constants ----------------
        iota_i = const.tile([P, C], I32)
        nc.gpsimd.iota(iota_i[:], pattern=[[1, C]], base=0, channel_multiplier=0)
        iota_row = const.tile([P, C], F32)
        nc.vector.tensor_copy(iota_row[:], iota_i[:])

        iotac_i = const.tile([C, 1], I32)
        nc.gpsimd.iota(iotac_i[:], pattern=[[0, 1]], base=0, channel_multiplier=1)
        iota_col = const.tile([C, 1], F32)
        nc.vector.tensor_copy(iota_col[:], iotac_i[:])

        ones_f = const.tile([1, C], F32)
        nc.vector.memset(ones_f[:1, :], 1.0)
        ones_row = const.tile([1, C], F32R)
        nc.vector.tensor_copy(ones_row[:1, :], ones_f[:1, :])

        crec_sb = const.tile([C, 1], F32)
        nc.sync.dma_start(out=crec_sb[:], in_=crec_d[:, :])

        centers_r = const.tile([C, D], F32R)
        centers_ext = const.tile([C, D + 2], F32R)

        iota_pi = const.tile([P, P], I32)
        nc.gpsimd.iota(iota_pi[:], pattern=[[1, P]], base=0, channel_multiplier=0)
        iota_pf = const.tile([P, P], F32)
        nc.vector.tensor_copy(iota_pf[:], iota_pi[:])
        iotac_pi = const.tile([P, 1], I32)
        nc.gpsimd.iota(iotac_pi[:], pattern=[[0, 1]], base=0, channel_multiplier=1)
        iotac_pf = const.tile([P, 1], F32)
        nc.vector.tensor_copy(iotac_pf[:], iotac_pi[:])
        ident_r = const.tile([P, P], F32R)
        nc.vector.tensor_scalar(
            out=ident_r[:],
            in0=iota_pf[:],
            scalar1=iotac_pf[:, :1],
            scalar2=None,
            op0=mybir.AluOpType.is_equal,
        )

        n_tiles_total = np_pad // P
        tp_pad = ((n_tiles_total + 511) // 512) * 512
        persist = ctx.enter_context(tc.tile_pool(name="persist", bufs=1))
        ttr_mode = p2var == "ttr"
        dot_mode = p2var == "dot" or ttr_mode
        sqf_all = persist.tile([P, tp_pad], F32, name="sqf_all") if dot_mode else None

        def emit_pass1():
            """Local segment sums -> sums_sb [C, D] (SBUF, f32)."""
            with tc.tile_pool(name="ps1", bufs=1, space="PSUM") as ps1:
                acc_ps = ps1.tile([C, D], F32, space="PSUM")
                ti = 0
                off = 0
                for g in groups:
                    w = g * P
                    feat_g = sb1.tile([P, GROUP * D], F32R, tag="feat1")
                    # partition p <- sample row off + t*128 + p
                    nc.sync.dma_start(
                        out=feat_g[:, : g * D].rearrange("p (t d) -> p t d", d=D),
                        in_=feat_d[off : off + w, :]
                        .rearrange("(p t) d -> p t d", p=P)
                        .bitcast(F32R),
                    )
                    labp_g = sb1.tile([P, GROUP], F32, tag="labp")
                    nc.sync.dma_start(
                        out=labp_g[:, :g],
                        in_=labp_d[off : off + w].rearrange("(p t) -> p t", p=P),
                    )
                    for t in range(g):
                        onehot = oh1.tile([P, C], F32R, tag="oh")
                        getattr(nc, oht1_eng).tensor_scalar(
                            out=onehot[:],
                            in0=iota_row[:],
                            scalar1=labp_g[:, t : t + 1],
                            scalar2=None,
                            op0=mybir.AluOpType.is_equal,
                        )
                        nc.tensor.matmul(
                            acc_ps[:],
                            lhsT=onehot[:],
                            rhs=feat_g[:, t * D : (t + 1) * D],
                            start=(ti == 0),
                            stop=(ti == n_tiles_total - 1),
                        )
                        if dot_mode and p1sq != "none":
                            sq1 = oh1.tile([P, D], F32, tag="sq1", bufs=dbufs)
                            if p1sq == "act" or (p1sq == "split" and ti % 2 == 1):
                                nc.scalar.activation(
                                    out=sq1[:],
                                    in_=feat_g[:, t * D : (t + 1) * D].bitcast(F32),
                                    func=mybir.ActivationFunctionType.Square,
                                    accum_out=sqf_all[:, ti : ti + 1],
                                )
                            else:
                                nc.vector.scalar_tensor_tensor(
                                    out=sq1[:],
                                    in0=feat_g[:, t * D : (t + 1) * D].bitcast(F32),
                                    scalar=1.0,
                                    in1=feat_g[:, t * D : (t + 1) * D].bitcast(F32),
                                    op0=mybir.AluOpType.mult,
                                    op1=mybir.AluOpType.mult,
                                    accum_out=sqf_all[:, ti : ti + 1],
                                )
                        ti += 1
                    off += w
                sums_sb = const.tile([C, D], F32)
                nc.vector.tensor_copy(sums_sb[:], acc_ps[:])
            return sums_sb

        def emit_centers(sums_sb, collective):
            """AllReduce sums (optional) and scale by 1/count -> centers_r."""
            cc_in = dram.tile([C, D], F32)
            nc.sync.dma_start(out=cc_in[:], in_=sums_sb[:])
            if collective:
                cc_out = dram.tile([C, D], F32)
                nc.gpsimd.collective_compute(
                    "AllReduce",
                    mybir.AluOpType.add,
                    replica_groups=[list(range(num_devices))],
                    ins=[cc_in.opt()],
                    outs=[cc_out.opt()],
                )
                gsrc = cc_out
            else:
                gsrc = cc_in
            gsums_sb = const.tile([C, D], F32)
            nc.sync.dma_start(out=gsums_sb[:], in_=gsrc[:])
            if dot_mode:
                cent_f = const.tile([C, D], F32)
                nc.vector.tensor_scalar(
                    out=cent_f[:],
                    in0=gsums_sb[:],
                    scalar1=crec_sb[:, :1],
                    scalar2=None,
                    op0=mybir.AluOpType.mult,
                )
                csq_scr = const.tile([C, D], F32)
                cext_f = const.tile([C, D + 2], F32)
                nc.vector.memset(cext_f[:, D : D + 2], 0.0)
                nc.scalar.activation(
                    out=csq_scr[:],
                    in_=cent_f[:],
                    func=mybir.ActivationFunctionType.Square,
                    accum_out=cext_f[:, D : D + 1],
                )
                nc.vector.tensor_copy(cext_f[:, :D], cent_f[:])
                nc.vector.tensor_copy(centers_ext[:], cext_f[:])
            else:
                nc.vector.tensor_scalar(
                    out=centers_r[:],
                    in0=gsums_sb[:],
                    scalar1=crec_sb[:, :1],
                    scalar2=None,
                    op0=mybir.AluOpType.mult,
                )
            return gsums_sb

        def emit_pass2():
            """Distances using centers_r -> out_d."""
            with (
                tc.tile_pool(name="ps_lb", bufs=2, space="PSUM") as ps_lb,
                tc.tile_pool(name="ps_g", bufs=gbufs, space="PSUM") as ps_g,
                tc.tile_pool(name="sb2", bufs=sb_bufs) as sb2,
                tc.tile_pool(name="resp", bufs=1) as resp,
            ):
                res_all = resp.tile([P, ((n_tiles_total + 511) // 512) * 512], F32)
                tbase = 0
                off = 0
                for g in groups:
                    w = g * P
                    f2dt = F32R if p2var in ("psum", "ps2") else F32
                    E = D + 2
                    feat_g = sb2.tile(
                        [P, GROUP * (E if ttr_mode else D)], f2dt, tag="feat2"
                    )
                    f2src = feat_d[off : off + w, :].rearrange("(p t) d -> p t d", p=P)
                    if ttr_mode:
                        fv = feat_g[:, : g * E].rearrange("p (t e) -> p t e", e=E)
                        nc.sync.dma_start(out=fv[:, :, :D], in_=f2src)
                        nc.vector.memset(fv[:, :, D:E], -0.5)
                    else:
                        nc.sync.dma_start(
                            out=feat_g[:, : g * D].rearrange("p (t d) -> p t d", d=D),
                            in_=f2src.bitcast(F32R) if p2var in ("psum", "ps2") else f2src,
                        )
                    labf_g = sb2.tile([1, GROUP * P], F32R, tag="labf")
                    if p2var != "fixedoht":
                        nc.sync.dma_start(
                            out=labf_g[:1, :w],
                            in_=labf_d[None, off : off + w].bitcast(F32R),
                        )
                    oht_g = sb2.tile([C, GROUP * P], F32R, tag="oht")
                    for h in ([] if p2var in ("nope", "fixedoht") else range(0, w, 512)):
                        hw = min(512, w - h)
                        lb_ps = ps_lb.tile([C, 512], F32, space="PSUM", tag="lb")
                        nc.tensor.matmul(
                            lb_ps[:, :hw],
                            lhsT=ones_row[:1, :],
                            rhs=labf_g[:1, h : h + hw],
                            start=True,
                            stop=True,
                        )
                        _oht_kw = (
                            dict(scalar2=-1.0, op1=mybir.AluOpType.mult)
                            if p2var in ("psum", "ps2")
                            else dict(scalar2=None)
                        )
                        if oht_eng == "gpsimd":
                            # gpsimd can't read PSUM: idle ACT stages lb to SBUF
                            lb_sb = sb2.tile([C, 512], F32, tag="lbsb", bufs=2)
                            nc.scalar.activation(
                                out=lb_sb[:, :hw],
                                in_=lb_ps[:, :hw],
                                func=mybir.ActivationFunctionType.Copy,
                            )
                            lb_src = lb_sb
                        else:
                            lb_src = lb_ps
                        getattr(nc, oht_eng).tensor_scalar(
                            out=oht_g[:, h : h + hw],
                            in0=lb_src[:, :hw],
                            scalar1=iota_col[:, :1],
                            op0=mybir.AluOpType.is_equal,
                            **_oht_kw,
                        )
                    res_g = res_all[:, tbase : tbase + g]
                    for t in range(g):
                        if ttr_mode:
                            # G_ext[p,:] = [centers[label_p,:], sqc[label_p]]
                            g_ps = ps_g.tile([P, E], F32, space="PSUM", tag="g")
                            nc.tensor.matmul(
                                g_ps[:],
                                lhsT=oht_g[:, t : w : g],
                                rhs=centers_ext[:],
                                start=True,
                                stop=True,
                            )
                            # accum = sum((-2 f) * g) = -2 f.c + sqc
                            scr = sb2.tile([P, E], F32, tag="scr", bufs=dbufs)
                            nc.vector.scalar_tensor_tensor(
                                out=scr[:],
                                in0=feat_g[:, t * E : (t + 1) * E],
                                scalar=-2.0,
                                in1=g_ps[:],
                                op0=mybir.AluOpType.mult,
                                op1=mybir.AluOpType.mult,
                                accum_out=res_g[:, t : t + 1],
                            )
                            continue
                        if p2var in ("psum", "ps2"):
                            # PSUM holds diff = feat - centers[label] via two
                            # accumulating matmuls (oht pre-scaled by -1).
                            g_ps = ps_g.tile([P, D], F32, space="PSUM", tag="g")
                            nc.tensor.matmul(
                                g_ps[:],
                                lhsT=oht_g[:, t : w : g],
                                rhs=centers_r[:],
                                start=True,
                                stop=False,
                            )
                            nc.tensor.matmul(
                                g_ps[:],
                                lhsT=ident_r[:],
                                rhs=feat_g[:, t * D : (t + 1) * D],
                                start=False,
                                stop=True,
                            )
                            if p2var == "ps2" and t % 2 == 1:
                                # odd tiles: DVE squares+reduces, ACT rests
                                scr = sb2.tile([P, D], F32, tag="scr", bufs=dbufs)
                                nc.vector.scalar_tensor_tensor(
                                    out=scr[:],
                                    in0=g_ps[:],
                                    scalar=1.0,
                                    in1=g_ps[:],
                                    op0=mybir.AluOpType.mult,
                                    op1=mybir.AluOpType.mult,
                                    accum_out=res_g[:, t : t + 1],
                                )
                            else:
                                nc.scalar.activation(
                                    out=g_ps[:],
                                    in_=g_ps[:],
                                    func=mybir.ActivationFunctionType.Square,
                                    accum_out=res_g[:, t : t + 1],
                                )
                            continue
                        if p2var == "fixedoht":
                            g_ps = ps_g.tile([P, D], F32, space="PSUM", tag="g")
                            nc.tensor.matmul(
                                g_ps[:],
                                lhsT=ident_r[:C, :],
                                rhs=centers_r[:],
                                start=True,
                                stop=True,
                            )
                            diff = sb2.tile([P, D], F32, tag="diff", bufs=dbufs)
                            nc.vector.tensor_tensor(
                                out=diff[:],
                                in0=feat_g[:, t * D : (t + 1) * D],
                                in1=g_ps[:],
                                op=mybir.AluOpType.subtract,
                            )
                            nc.scalar.activation(
                                out=diff[:],
                                in_=diff[:],
                                func=mybir.ActivationFunctionType.Square,
                                accum_out=res_g[:, t : t + 1],
                            )
                            continue
                        use_g = p2var in ("std", "noact")
                        if use_g:
                            g_ps = ps_g.tile([P, D], F32, space="PSUM", tag="g")
                            nc.tensor.matmul(
                                g_ps[:],
                                lhsT=oht_g[:, t : w : g],
                                rhs=centers_r[:],
                                start=True,
                                stop=True,
                            )
                        if p2var in ("std", "noact"):
                            diff = sb2.tile([P, D], F32, tag="diff", bufs=dbufs)
                            nc.vector.tensor_tensor(
                                out=diff[:],
                                in0=feat_g[:, t * D : (t + 1) * D],
                                in1=g_ps[:],
                                op=mybir.AluOpType.subtract,
                            )
                        if p2var == "std":
                            nc.scalar.activation(
                                out=diff[:],
                                in_=diff[:],
                                func=mybir.ActivationFunctionType.Square,
                                accum_out=res_g[:, t : t + 1],
                            )
                        elif p2var == "noact":
                            nc.vector.tensor_copy(res_g[:, t : t + 1], diff[:, 0:1])
                        elif p2var == "nodve":
                            sq = sb2.tile([P, D], F32, tag="sq")
                            nc.scalar.activation(
                                out=sq[:],
                                in_=feat_g[:, t * D : (t + 1) * D],
                                func=mybir.ActivationFunctionType.Square,
                                accum_out=res_g[:, t : t + 1],
                            )
                        elif p2var == "nope":
                            sq = sb2.tile([P, D], F32, tag="sq")
                            nc.scalar.activation(
                                out=sq[:],
                                in_=feat_g[:, t * D : (t + 1) * D],
                                func=mybir.ActivationFunctionType.Square,
                                accum_out=res_g[:, t : t + 1],
                            )
                    tbase += g
                    off += w
                # single sqrt over every accumulated column, then write out.
                if dot_mode:
                    nc.vector.tensor_tensor(
                        out=res_all[:, :n_tiles_total],
                        in0=res_all[:, :n_tiles_total],
                        in1=sqf_all[:, :n_tiles_total],
                        op=mybir.AluOpType.add,
                    )
                if ttr_mode:
                    nc.vector.tensor_scalar_max(
                        out=res_all[:, :n_tiles_total],
                        in0=res_all[:, :n_tiles_total],
                        scalar1=0.0,
                    )
                nc.scalar.activation(
                    out=res_all[:, :n_tiles_total],
                    in_=res_all[:, :n_tiles_total],
                    func=mybir.ActivationFunctionType.Sqrt,
                )
                # uniform groups: out[goff + p*g + t] = res_all[p, 8*grp + t]
                n_uni = np_pad // (GROUP * P)
                if n_uni:
                    nc.sync.dma_start(
                        out=out_d[: n_uni * GROUP * P].rearrange(
                            "(grp p t) -> p grp t", p=P, t=GROUP
                        ),
                        in_=res_all[:, : n_uni * GROUP].rearrange(
                            "p (grp t) -> p grp t", t=GROUP
                        ),
                    )
                toff = n_uni * GROUP
                soff = n_uni * GROUP * P
                for g in groups[n_uni:]:
                    nc.sync.dma_start(
                        out=out_d[soff : soff + g * P].rearrange("(p t) -> p t", p=P),
                        in_=res_all[:, toff : toff + g],
                    )
                    toff += g
                    soff += g * P

        if loops:
            if loop_what in ("all", "p2") and not do_p1:
                cfill = const.tile([C, D + 2], F32)
                nc.vector.memset(cfill[:], 0.01)
                nc.vector.tensor_copy(centers_r[:], cfill[:, :D])
                nc.vector.tensor_copy(centers_ext[:], cfill[:])
                if dot_mode:
                    nc.vector.memset(sqf_all[:], 0.0)
            with tc.For_i(0, loops, 1):
                if loop_what == "dma":
                    off = 0
                    for g in groups:
                        w = g * P
                        feat_g = sb1.tile([P, GROUP * D], F32, tag="featd")
                        nc.sync.dma_start(
                            out=feat_g[:, : g * D].rearrange("p (t d) -> p t d", d=D),
                            in_=feat_d[off : off + w, :].rearrange(
                                "(p t) d -> p t d", p=P
                            ),
                        )
                        off += w
                if do_p1:
                    sums_sb = emit_pass1()
                    emit_centers(sums_sb, collective=False)
                if do_p2:
                    emit_pass2()
            if loop_what == "dma":
                z = const.tile([P, 1], F32)
                nc.vector.memset(z[:], 0.0)
                nc.sync.dma_start(out=out_d[0:P, None], in_=z[:])
        elif mode == "dma":
            for _ in range(2):
                off = 0
                for g in groups:
                    w = g * P
                    feat_g = sb1.tile([P, GROUP * D], F32, tag="featd")
                    nc.sync.dma_start(
                        out=feat_g[:, : g * D].rearrange("p (t d) -> p t d", d=D),
                        in_=feat_d[off : off + w, :].rearrange("(p t) d -> p t d", p=P),
                    )
                    off += w
            z = const.tile([P, 1], F32)
            nc.vector.memset(z[:], 0.0)
            nc.sync.dma_start(out=out_d[0:P, None], in_=z[:])
        else:
            if do_p1:
                sums_sb = emit_pass1()
                gsums_sb = emit_centers(sums_sb, collective=do_cc)
                if mode in ("pass1", "pass1nc"):
                    nc.sync.dma_start(
                        out=out_d[0 : C * D].rearrange("(c d) -> c d", d=D),
                        in_=gsums_sb[:],
                    )
            elif do_p2:
                cfill = const.tile([C, D + 2], F32)
                nc.vector.memset(cfill[:], 0.01)
                nc.vector.tensor_copy(centers_r[:], cfill[:, :D])
                nc.vector.tensor_copy(centers_ext[:], cfill[:])
                if dot_mode:
                    nc.vector.memset(sqf_all[:], 0.0)
            if do_p2:
                emit_pass2()

    nc.compile()
    return nc


def build_nop(num_devices=N_CORES):
    """Minimal kernel (copy one tile) to measure the dispatch floor."""
    nc = bacc.Bacc(
        "TRN2",
        target_bir_lowering=False,
        debug=False,
        enable_asserts=True,
        num_devices=num_devices,
    )
    x_d = nc.dram_tensor("x", [P, P], F32, kind="ExternalInput")
    y_d = nc.dram_tensor("y", [P, P], F32, kind="ExternalOutput")
    with tile.TileContext(nc) as tc:
        with tc.tile_pool(name="sb", bufs=1) as sb:
            t = sb.tile([P, P], F32)
            nc.sync.dma_start(out=t[:], in_=x_d[:, :])
            nc.sync.dma_start(out=y_d[:, :], in_=t[:])
    nc.compile()
    return nc


def _prep_core_inputs(feat_c, lab_c, crec, np_pad):
    """Host-side shard prep: pad + layout labels; all f32."""
    ns = feat_c.shape[0]
    fpad = np.zeros((np_pad, D), dtype=np.float32)
    fpad[:ns] = feat_c
    lab = np.full((np_pad,), float(C), dtype=np.float32)  # pad label = C -> no class
    lab[:ns] = lab_c.astype(np.float32)
    return {"feat": fpad, "labp": lab, "labf": lab, "crec": crec}


_CACHE = {}


def _get_nc(np_pad, num_devices, **kw):
    key = (np_pad, num_devices, tuple(sorted(kw.items())))
    if key not in _CACHE:
        _CACHE[key] = build(np_pad, num_devices, **kw)
    return _CACHE[key]


def run(feat, label, np_pad=None, num_devices=N_CORES, trace=False, **kw):
    n = feat.shape[0]
    ns = n // num_devices
    if np_pad is None:
        np_pad = ((ns + P - 1) // P) * P
    nc = _get_nc(np_pad, num_devices, **kw)

    cnt = np.bincount(label.astype(np.int64), minlength=C)[:C]
    crec = (1.0 / np.maximum(cnt, 1)).astype(np.float32)[:, None]

    in_maps = [
        _prep_core_inputs(
            feat[c * ns : (c + 1) * ns], label[c * ns : (c + 1) * ns], crec, np_pad
        )
        for c in range(num_devices)
    ]
    res = run_bass_kernel_spmd(
        nc, in_maps, core_ids=list(range(num_devices)), trace=trace
    )
    out = np.concatenate([res.results[c]["given"][:ns] for c in range(num_devices)])
    return out, res


def kernel(feat, label):
    feat = np.asarray(feat, dtype=np.float32)
    label = np.asarray(label)
    out, _ = run(feat, label)
    return out.astype(np.float32)

